# revision 1
# baseline (speedup 1.0000x reference)
"""Trainium2 Bass kernel for nn_AttentionFusion (dense_transformer).

Reference computation per batch element b (B=8 -> one NeuronCore each):
    w_ds = bilinear_downsample(feat_wide[b], 4)   # [C,64,64], exact 2x2 avg at (4i+1..4i+2)
    n_ds = bilinear_downsample(feat_narrow[b], 4)
    Q = w_ds.reshape(C, N); K = n_ds.reshape(C, N)    # N = 4096
    attn = softmax(Q^T K / sqrt(C), axis=-1)          # [N, N]
    out_small = (attn @ K^T)^T                        # [C, N]
    out = feat_wide[b] + bilinear_upsample(out_small.reshape(C,64,64), 4)

Mapping to the hardware (all on-chip after the DMA loads):
  - downsample: strided DMA of rows 4i+1,4i+2 only + DVE adds -> q_bf/k_bf bf16
    [128, 4096]; the 0.25 average scale is folded into the exp scale (1/16) and
    the W-upsample constant (1/4), so the adds are unscaled.
  - K^T (+ ones column for the softmax row-sum) via PE transposes -> kt1
  - scores^T per (m-tile, n-block): PE matmul lhsT=K-tile rhs=Q-block; the PV
    matmuls of the previous n-block are interleaved in program order so the PE
    stays dense while ScalarE runs the exps.
  - exp on ScalarE (scale folded in), written as bf16 attn^T tiles
  - PV: PE matmul lhsT=attnT-tile rhs=kt1-tile accumulating over m; the ones
    column yields the softmax denominator per partition; normalize with DVE
    reciprocal + tensor_scalar
  - W-upsample: PE matmul with a constant [128, 512] block-diagonal weight
  - H-upsample + residual: DVE scalar_tensor_tensor pairs, computed in place
    on the streamed feat_wide row blocks
"""

import math

import numpy as np


# ----------------------------------------------------------------------------
# numpy-side constants
# ----------------------------------------------------------------------------

def _build_upsample_matrix(n_in: int, n_out: int) -> np.ndarray:
    """U[h, H]: out[H] = sum_h U[h, H] * in[h] for torch-style bilinear,
    align_corners=False, antialias=False, scale n_out/n_in."""
    U = np.zeros((n_in, n_out), dtype=np.float64)
    scale = n_in / n_out
    for o in range(n_out):
        src = (o + 0.5) * scale - 0.5
        k0 = int(math.floor(src))
        frac = src - k0
        for k, wt in ((k0, 1.0 - frac), (k0 + 1, frac)):
            kc = min(max(k, 0), n_in - 1)
            U[kc, o] += wt
    return U


def _build_uw_block() -> np.ndarray:
    """[128, 512] block-diag W-upsample weights (two 64->256 blocks), pre-scaled
    by 1/4 to undo the unscaled 2x2-average downsample of K."""
    U = _build_upsample_matrix(64, 256) * 0.25
    blk = np.zeros((128, 512), dtype=np.float64)
    blk[0:64, 0:256] = U
    blk[64:128, 256:512] = U
    return blk


# ----------------------------------------------------------------------------
# Bass kernel builder
# ----------------------------------------------------------------------------

def build_kernel():
    import concourse.bacc as bacc
    import concourse.bass as bass
    import concourse.mybir as mybir
    from concourse import tile

    f32 = mybir.dt.float32
    bf16 = mybir.dt.bfloat16
    AOp = mybir.AluOpType
    ActFn = mybir.ActivationFunctionType

    C = 128          # channels = partitions
    HW = 256         # full resolution
    hw = 64          # downsampled resolution
    N = hw * hw      # 4096 attention positions
    MI = 32          # m tiles of 128
    NBLK = 8         # n blocks of 512
    NSUB = 4         # n sub-tiles of 128 per block
    # scores = (4Q)^T (4K) / (16 sqrt(C)); the ds 2x2 sums are unscaled
    EXP_SCALE = 1.0 / (16.0 * math.sqrt(C))

    nc = bacc.Bacc("TRN2", target_bir_lowering=False, debug=False)

    fw = nc.dram_tensor("feat_wide", [C, HW, HW], f32, kind="ExternalInput")
    fn = nc.dram_tensor("feat_narrow", [C, HW, HW], f32, kind="ExternalInput")
    uw = nc.dram_tensor("uwblk", [128, 512], bf16, kind="ExternalInput")
    ident = nc.dram_tensor("ident", [128, 128], bf16, kind="ExternalInput")
    out = nc.dram_tensor("out", [C, HW, HW], f32, kind="ExternalOutput")

    with tile.TileContext(nc) as tc:
        with (
            tc.tile_pool(name="const", bufs=1) as const_pool,
            tc.tile_pool(name="qk", bufs=1) as qk_pool,
            tc.tile_pool(name="ds", bufs=3) as ds_pool,
            tc.tile_pool(name="io", bufs=3) as io_pool,
            tc.tile_pool(name="attn", bufs=2) as attn_pool,
            tc.tile_pool(name="small", bufs=3) as small_pool,
            tc.tile_pool(name="dstmp", bufs=3) as dstmp_pool,
            tc.tile_pool(name="ps_s", bufs=2, space=bass.MemorySpace.PSUM) as ps_s,
            tc.tile_pool(name="ps_o", bufs=3, space=bass.MemorySpace.PSUM) as ps_o,
            tc.tile_pool(name="ps_y", bufs=1, space=bass.MemorySpace.PSUM) as ps_y,
        ):
            # ---- constants ----
            uw_t = const_pool.tile([128, 512], bf16)
            nc.sync.dma_start(uw_t[:], uw[:, :])
            id_t = const_pool.tile([128, 128], bf16)
            nc.sync.dma_start(id_t[:], ident[:, :])

            # ---- downsample: feat -> q_bf / k_bf  [128, 4096] bf16 (4x scale) ----
            q_bf = qk_pool.tile([C, N], bf16)
            k_bf = qk_pool.tile([C, N], bf16)
            kt1 = qk_pool.tile([128, MI, 129], bf16)
            nc.vector.memset(kt1[:], 1.0)

            DS_I = 4  # i-rows per chunk
            NCH = hw // DS_I

            def emit_ds_chunk(src, dst, cc):
                src3 = src.ap().rearrange("c (i r) w -> c i (r w)", r=4)
                dst3 = dst[:].rearrange("c (i w) -> c i w", w=hw)
                i0 = cc * DS_I
                ch = ds_pool.tile([C, DS_I, 512], f32, tag="ch")
                # rows 4i+1, 4i+2 are adjacent -> 2KB contiguous lines
                nc.sync.dma_start(ch[:], src3[:, i0 : i0 + DS_I, 256:768])
                ch5 = ch[:].rearrange("c i (r k f) -> c i r k f", r=2, f=4)
                # ds = sum of the 4 center samples (scale folded downstream)
                a = dstmp_pool.tile([C, DS_I, hw], f32, tag="dsa")
                b = dstmp_pool.tile([C, DS_I, hw], f32, tag="dsb")
                nc.vector.tensor_tensor(
                    a[:], ch5[:, :, 0, :, 1], ch5[:, :, 1, :, 1], AOp.add
                )
                nc.vector.tensor_tensor(
                    b[:], ch5[:, :, 0, :, 2], ch5[:, :, 1, :, 2], AOp.add
                )
                nc.vector.tensor_tensor(
                    dst3[:, i0 : i0 + DS_I, :], a[:], b[:], AOp.add
                )
                if dst is k_bf:
                    # K^T tiles for this chunk (with ones column preset above)
                    for mi in (2 * cc, 2 * cc + 1):
                        pt = ps_o.tile([128, 128], bf16, tag="po")
                        nc.tensor.transpose(
                            pt[:], k_bf[:, mi * 128 : (mi + 1) * 128], id_t[:]
                        )
                        nc.vector.tensor_copy(kt1[:, mi, 0:128], pt[:])

            # q0/q1 first (block-0 scores need them), then k chunks streaming
            # (each scores m-pair only needs its own k chunk), with enough q
            # chunks slotted in to keep ahead of the block cadence
            order = [(fw, q_bf, 0), (fw, q_bf, 1)]
            qs = 2
            for cc in range(NCH):
                order.append((fn, k_bf, cc))
                if cc % 4 == 3 and qs < 6:  # q2..q5 after k3/k7/k11/k15
                    order.append((fw, q_bf, qs))
                    qs += 1
            for cc in range(qs, NCH):
                order.append((fw, q_bf, cc))
            for src, dst, cc in order:
                emit_ds_chunk(src, dst, cc)

            # ---- y = W-upsampled attention output [128, 64, 256] bf16 ----
            y = qk_pool.tile([C, hw, HW], bf16)
            y3 = y[:]  # [128, 64, 256]

            # ---- attention: interleave scores of block nb with PV of nb-1 ----
            at_tiles = {}

            def emit_scores(nb, mp):
                """scores^T + exp for m-pair mp of n-block nb."""
                at = at_tiles[nb]
                ps = ps_s.tile([128, 2, 512], f32, tag="ps")
                for s in range(2):
                    mi = 2 * mp + s
                    nc.tensor.matmul(
                        ps[:, s, :],
                        k_bf[:, mi * 128 : (mi + 1) * 128],
                        q_bf[:, nb * 512 : (nb + 1) * 512],
                        start=True,
                        stop=True,
                    )
                nc.scalar.activation(
                    at[:, 2 * mp : 2 * mp + 2, :],
                    ps[:],
                    ActFn.Exp,
                    bias=0.0,
                    scale=EXP_SCALE,
                )

            def emit_pv_mms(nb, ns, po, mi0, mi1):
                """PV matmul chunk [mi0, mi1) for n-sub-tile ns of block nb."""
                at = at_tiles[nb]
                for mi in range(mi0, mi1):
                    nc.tensor.matmul(
                        po[:],
                        at[:, mi, ns * 128 : (ns + 1) * 128],
                        kt1[:, mi, :],
                        start=(mi == 0),
                        stop=(mi == MI - 1),
                    )

            def emit_pv_tail(nb, ns, po):
                """normalize + W-up for n-sub-tile ns of n-block nb."""
                t = nb * NSUB + ns  # global n-tile (2 h-rows)
                rcp = small_pool.tile([128, 1], f32, tag="rcp")
                nc.vector.reciprocal(rcp[:], po[:, 128:129])
                ot = small_pool.tile([128, 128], bf16, tag="ot")
                nc.vector.tensor_scalar(ot[:], po[:, 0:128], rcp[:], None, AOp.mult)
                py = ps_y.tile([128, 512], f32, tag="py")
                nc.tensor.matmul(py[:], ot[:], uw_t[:], start=True, stop=True)
                nc.scalar.copy(y3[:, 2 * t : 2 * t + 2, :], py[:])

            for nb in range(NBLK + 1):
                if nb < NBLK:
                    at = attn_pool.tile([128, MI, 512], bf16, tag="at")
                    at_tiles[nb] = at
                # fine interleave: one scores m-pair (2 MMs + exp), then 8 PV
                # MMs of the previous block -- PV work hides the exp latency.
                # PV tails are batched per 2 sub-tiles so a stalled y-copy on
                # ScalarE doesn't sit in front of the next exps.
                pos = {}
                for ns in range(NSUB):
                    po = None
                    if nb > 0:
                        po = ps_o.tile([128, 129], f32, tag="po")
                        pos[ns] = po
                    for i in range(4):
                        if nb < NBLK:
                            emit_scores(nb, 4 * ns + i)
                        if po is not None:
                            emit_pv_mms(nb - 1, ns, po, 8 * i, 8 * i + 8)
                    if po is not None and ns % 2 == 1:
                        emit_pv_tail(nb - 1, ns - 1, pos[ns - 1])
                        emit_pv_tail(nb - 1, ns, pos[ns])

            # ---- H-upsample + residual, in place on streamed fw row blocks ----
            # out[4k+r] = wa[r]*y[k+d[r]] + wb[r]*y[k+d[r]+1] + fw[4k+r]
            PH = (
                (0.375, 0.625, -1),
                (0.125, 0.875, -1),
                (0.875, 0.125, 0),
                (0.625, 0.375, 0),
            )
            KB = 4  # y rows per block -> 16 output rows
            NKB = hw // KB
            for kb in range(NKB):
                k0 = kb * KB
                h0 = 4 * k0
                fwb = io_pool.tile([C, 4 * KB, HW], f32, tag="io")
                nc.gpsimd.dma_start(fwb[:], fw.ap()[:, h0 : h0 + 4 * KB, :])
                fw4 = fwb[:].rearrange("c (j r) w -> c j r w", r=4)
                for r, (wa, wb, d) in enumerate(PH):
                    js, je = 0, KB
                    if kb == 0 and d == -1:
                        js = 1
                    if kb == NKB - 1 and d == 0:
                        je = KB - 1
                    # edge rows: clamped -> out = 1.0*y[edge] + fw
                    if js == 1:
                        nc.vector.scalar_tensor_tensor(
                            fw4[:, 0, r, :], y3[:, 0, :], 1.0,
                            fw4[:, 0, r, :], AOp.mult, AOp.add,
                        )
                    if je == KB - 1:
                        nc.vector.scalar_tensor_tensor(
                            fw4[:, KB - 1, r, :], y3[:, hw - 1, :], 1.0,
                            fw4[:, KB - 1, r, :], AOp.mult, AOp.add,
                        )
                    cnt = je - js
                    ka = k0 + js + d
                    tm = small_pool.tile([C, KB, HW], bf16, tag="tm")
                    # tm = (wa/wb) * y[ka..] + y[ka+1..]       (all bf16)
                    nc.vector.scalar_tensor_tensor(
                        tm[:, 0:cnt, :], y3[:, ka : ka + cnt, :], wa / wb,
                        y3[:, ka + 1 : ka + 1 + cnt, :], AOp.mult, AOp.add,
                    )
                    # fw_rows += wb * tm   (in place)
                    nc.vector.scalar_tensor_tensor(
                        fw4[:, js:je, r, :], tm[:, 0:cnt, :], wb,
                        fw4[:, js:je, r, :], AOp.mult, AOp.add,
                    )
                nc.sync.dma_start(out.ap()[:, h0 : h0 + 4 * KB, :], fwb[:])

    nc.compile()
    return nc


_NC_CACHE = None


def _get_nc():
    global _NC_CACHE
    if _NC_CACHE is None:
        _NC_CACHE = build_kernel()
    return _NC_CACHE


def run(feat_wide: np.ndarray, feat_narrow: np.ndarray, trace: bool = False):
    """Run on 8 NeuronCores; returns (output [8,128,256,256], BassKernelResults)."""
    from concourse.bass_utils import run_bass_kernel_spmd
    import ml_dtypes

    B, C, H, W = feat_wide.shape
    assert (B, C, H, W) == (8, 128, 256, 256)

    uwblk = _build_uw_block().astype(ml_dtypes.bfloat16)
    identity = np.eye(128, dtype=ml_dtypes.bfloat16)

    nc = _get_nc()
    in_maps = [
        {
            "feat_wide": np.ascontiguousarray(np.asarray(feat_wide[b], dtype=np.float32)),
            "feat_narrow": np.ascontiguousarray(np.asarray(feat_narrow[b], dtype=np.float32)),
            "uwblk": uwblk,
            "ident": identity,
        }
        for b in range(B)
    ]
    res = run_bass_kernel_spmd(nc, in_maps, core_ids=list(range(8)), trace=trace)
    out = np.stack([res.results[b]["out"] for b in range(B)], axis=0)
    return out, res


def kernel(feat_wide: np.ndarray, feat_narrow: np.ndarray) -> np.ndarray:
    out, _ = run(feat_wide, feat_narrow, trace=False)
    return out



# revision 2
# speedup vs baseline: 1.0049x; 1.0049x over previous
"""Trainium2 Bass kernel v4 for nn_AttentionFusion — fused stream, PE upsample.

Per batch element (B=8 -> one NeuronCore each):
    Q = ds(feat_wide), K = ds(feat_narrow)      # 2x2 center sums, [C, 4096]
    attn = softmax(Q^T K / (16 sqrt(C)))
    out = feat_wide + up4((attn @ K^T)^T)       # bilinear 4x upsample + residual

v4 layout: feat_wide streams through SBUF ONCE in 16-row blocks (80 MiB total
HBM traffic).  The whole upsample runs on the PE: for each normalized
attention sub-tile ot[tt] = [n=(2h,64w), c] the output rows are produced by
matmuls against 8 host-built [128, 512] matrices that fuse the 2-tap H-up
weights with the W-upsample matrix; bands of 2 output rows accumulate in PSUM
(cross-tile bands get one matmul from each neighbor tile), then a single DVE
add folds the streamed feat_wide rows in-place and the block is written out.

Engine mapping per n-block iteration (~22 us DMA slot):
  - PE: PV(nb-1) m-region-wise ahead of scores(nb) (single fp8 attn buffer,
    WAR-safe), band matmuls of tiles from nb-2 interleaved between regions
  - ScalarE: exp only (PSUM -> fp8 SBUF)
  - DVE: normalize (rcp + scale) + one [128,512] band add per 2 output rows
  - Pool: fw DMA waits + Q extraction (row-pair + col-pair sums)
  - Sync: fn chunk loads (prologue), fw block loads, out writes
"""

import math

import numpy as np


# ----------------------------------------------------------------------------
# numpy-side constants
# ----------------------------------------------------------------------------

def _build_upsample_matrix(n_in: int, n_out: int) -> np.ndarray:
    """U[h, H]: out[H] = sum_h U[h, H] * in[h] for torch-style bilinear,
    align_corners=False, antialias=False, scale n_out/n_in."""
    U = np.zeros((n_in, n_out), dtype=np.float64)
    scale = n_in / n_out
    for o in range(n_out):
        src = (o + 0.5) * scale - 0.5
        k0 = int(math.floor(src))
        frac = src - k0
        for k, wt in ((k0, 1.0 - frac), (k0 + 1, frac)):
            kc = min(max(k, 0), n_in - 1)
            U[kc, o] += wt
    return U


# Band weight matrices: band m covers output rows (2m, 2m+1).
#   m even (=2k): rows 4k+0,4k+1 = (0.375, 0.125) * y[k-1] + (0.625, 0.875) * y[k]
#   m odd (=2k+1): rows 4k+2,4k+3 = (0.875, 0.625) * y[k] + (0.125, 0.375) * y[k+1]
# y[k] lives in ot tile k//2 at h' = k%2.  Matrix M[(h',w), (r2,W)] =
# coef[h'][r2] * Uw[w, W] with Uw = 0.25 * U(64->256) (0.25 undoes the
# unscaled 2x2-sum downsample of K).
_BAND_COEFS = {
    "IN0": {0: (0.375, 0.125), 1: (0.625, 0.875)},  # m even, k odd: in-tile
    "IN1": {0: (0.875, 0.625), 1: (0.125, 0.375)},  # m odd, k even: in-tile
    "CEA": {1: (0.375, 0.125)},  # m even, k even: y[k-1] from tile tt-1
    "CEB": {0: (0.625, 0.875)},  # m even, k even: y[k] from tile tt
    "COA": {1: (0.875, 0.625)},  # m odd, k odd: y[k] from tile tt-1
    "COB": {0: (0.125, 0.375)},  # m odd, k odd: y[k+1] from tile tt
    "ET": {0: (1.0, 1.0)},       # band 0: rows 0,1 clamp to y[0]
    "EB": {1: (1.0, 1.0)},       # band 127: rows 254,255 clamp to y[63]
}
_BAND_NAMES = ("IN0", "IN1", "CEA", "CEB", "COA", "COB", "ET", "EB")


def _build_band_matrices() -> np.ndarray:
    """[8, 128, 512] f64: fused H-up x W-up band weight matrices."""
    Uw = _build_upsample_matrix(64, 256) * 0.25  # [64, 256]
    out = np.zeros((8, 128, 512), dtype=np.float64)
    for idx, name in enumerate(_BAND_NAMES):
        for hp, (c0, c1) in _BAND_COEFS[name].items():
            for r2, cf in ((0, c0), (1, c1)):
                out[idx, hp * 64 : hp * 64 + 64, r2 * 256 : r2 * 256 + 256] = (
                    cf * Uw
                )
    return out


# ----------------------------------------------------------------------------
# Bass kernel builder
# ----------------------------------------------------------------------------

def build_kernel():
    import concourse.bacc as bacc
    import concourse.bass as bass
    import concourse.mybir as mybir
    from concourse import tile

    f32 = mybir.dt.float32
    bf16 = mybir.dt.bfloat16
    f8 = mybir.dt.float8e4
    AOp = mybir.AluOpType
    ActFn = mybir.ActivationFunctionType

    C = 128          # channels = partitions
    HW = 256         # full resolution
    hw = 64          # downsampled resolution
    N = hw * hw      # 4096 attention positions
    MI = 32          # m tiles of 128
    NBLK = 8         # n blocks of 512
    NFB = 16         # fw row blocks of 16 rows
    EXP_SCALE = 1.0 / (16.0 * math.sqrt(C))
    MIDX = {n: i for i, n in enumerate(_BAND_NAMES)}

    nc = bacc.Bacc("TRN2", target_bir_lowering=False, debug=False)

    fw = nc.dram_tensor("feat_wide", [C, HW, HW], f32, kind="ExternalInput")
    fn = nc.dram_tensor("feat_narrow", [C, HW, HW], f32, kind="ExternalInput")
    bm = nc.dram_tensor("bandmat", [8, 128, 512], bf16, kind="ExternalInput")
    ident = nc.dram_tensor("ident", [128, 128], bf16, kind="ExternalInput")
    out = nc.dram_tensor("out", [C, HW, HW], f32, kind="ExternalOutput")

    with tile.TileContext(nc) as tc:
        with (
            tc.tile_pool(name="const", bufs=1) as const_pool,
            tc.tile_pool(name="big", bufs=1) as big_pool,
            tc.tile_pool(name="fwp", bufs=8) as fw_pool,
            tc.tile_pool(name="kch", bufs=2) as kch_pool,
            tc.tile_pool(name="qb", bufs=2) as qb_pool,
            tc.tile_pool(name="rs", bufs=2) as rs_pool,
            tc.tile_pool(name="sm", bufs=4) as sm_pool,
            tc.tile_pool(name="ps_s", bufs=2, space=bass.MemorySpace.PSUM) as ps_s,
            tc.tile_pool(name="ps_o", bufs=2, space=bass.MemorySpace.PSUM) as ps_o,
            tc.tile_pool(name="ps_b", bufs=2, space=bass.MemorySpace.PSUM) as ps_b,
        ):
            # ---- constants ----
            bm_t = const_pool.tile([128, 8, 512], bf16)
            for j in range(8):
                nc.sync.dma_start(bm_t[:, j, :], bm.ap()[j, :, :])
            id_t = const_pool.tile([128, 128], bf16)
            nc.sync.dma_start(id_t[:], ident[:, :])

            # ---- persistent buffers ----
            k_bf = big_pool.tile([C, N], bf16)
            kt1 = big_pool.tile([128, MI, 129], f8)   # K^T tiles + ones col
            nc.vector.memset(kt1[:], 1.0)
            at = big_pool.tile([128, MI, 512], f8)    # attn^T, single buffer

            fn3 = fn.ap().rearrange("c (i r) w -> c i (r w)", r=4)

            fw_tiles = {}
            q_tiles = {}
            ot_tiles = {}

            def load_fw(b):
                """Stream in fw rows 16b..16b+16 (sync queue dispatch)."""
                t = fw_pool.tile([C, 16, HW], f32, tag="fw")
                nc.sync.dma_start(t[:], fw.ap()[:, 16 * b : 16 * b + 16, :])
                fw_tiles[b] = t

            def q_extract(b):
                """Q columns of fw block b: rows 4i+1,4i+2, cols 4j+1,4j+2.
                Entirely on Pool so the DMA wait never blocks DVE."""
                t = fw_tiles[b]
                nbq, half = divmod(b, 2)
                if half == 0:
                    q_tiles[nbq] = qb_pool.tile([C, 512], bf16, tag="q", name="qt")
                qt = q_tiles[nbq]
                fw4 = t[:].rearrange("c (j r) w -> c j r w", r=4)
                rsum = rs_pool.tile([C, 4, 256], bf16, tag="qrs")
                nc.gpsimd.tensor_tensor(
                    rsum[:], fw4[:, :, 1, :], fw4[:, :, 2, :], AOp.add
                )
                rs4 = rsum[:].rearrange("c j (k f) -> c j k f", f=4)
                qv = qt[:, half * 256 : half * 256 + 256].rearrange(
                    "c (j k) -> c j k", k=hw
                )
                nc.gpsimd.tensor_tensor(qv, rs4[:, :, :, 1], rs4[:, :, :, 2], AOp.add)

            def load_k_chunk(cc):
                """fn rows 4i+1,4i+2 for i=4cc..4cc+4 -> k_bf cols + kt1 tiles."""
                ch = kch_pool.tile([C, 4, 512], f32, tag="kch")
                nc.sync.dma_start(ch[:], fn3[:, 4 * cc : 4 * cc + 4, 256:768])
                ch2 = ch[:].rearrange("c i (r w) -> c i r w", r=2)
                rsum = rs_pool.tile([C, 4, 256], bf16, tag="krs")
                nc.vector.tensor_tensor(rsum[:], ch2[:, :, 0, :], ch2[:, :, 1, :], AOp.add)
                rs4 = rsum[:].rearrange("c j (k f) -> c j k f", f=4)
                kv = k_bf[:, 256 * cc : 256 * cc + 256].rearrange(
                    "c (j k) -> c j k", k=hw
                )
                nc.vector.tensor_tensor(kv, rs4[:, :, :, 1], rs4[:, :, :, 2], AOp.add)
                for mi in (2 * cc, 2 * cc + 1):
                    pt = ps_b.tile([128, 512], f32, tag="bd")
                    ptb = pt[:, 0:64].bitcast(bf16)  # [128, 128] bf16 view
                    nc.tensor.transpose(
                        ptb, k_bf[:, 128 * mi : 128 * mi + 128], id_t[:]
                    )
                    nc.vector.tensor_copy(kt1[:, mi, 0:128], ptb)

            # ---- prologue: fw first (q gates the first scores), then K ----
            load_fw(0)
            load_fw(1)
            for cc in range(16):
                load_k_chunk(cc)
            q_extract(0)
            q_extract(1)

            def emit_band(m):
                """Band m = output rows (2m, 2m+1): PE matmul(s) into PSUM,
                DVE add of streamed fw rows in place, write block when done."""
                tt = None  # owning tail tile for bookkeeping only
                bd = ps_b.tile([128, 512], f32, tag="bd")
                if m == 0:
                    mms = ((0, "ET"),)
                elif m == 127:
                    mms = ((31, "EB"),)
                elif m % 2 == 0:
                    k = m // 2
                    if k % 2 == 1:
                        mms = (((k - 1) // 2, "IN0"),)
                    else:
                        mms = ((k // 2 - 1, "CEA"), (k // 2, "CEB"))
                else:
                    k = (m - 1) // 2
                    if k % 2 == 0:
                        mms = ((k // 2, "IN1"),)
                    else:
                        mms = (((k - 1) // 2, "COA"), ((k + 1) // 2, "COB"))
                n_mm = len(mms)
                for j, (tt, name) in enumerate(mms):
                    nc.tensor.matmul(
                        bd[:],
                        ot_tiles[tt][:],
                        bm_t[:, MIDX[name], :],
                        start=(j == 0),
                        stop=(j == n_mm - 1),
                    )
                b = m // 8
                lr = 2 * (m % 8)
                fwb = fw_tiles[b]
                nc.vector.tensor_tensor(
                    fwb[:, lr : lr + 2, :], bd[:], fwb[:, lr : lr + 2, :], AOp.add
                )
                if m % 8 == 7:
                    nc.sync.dma_start(
                        out.ap()[:, 16 * b : 16 * b + 16, :], fw_tiles.pop(b)[:]
                    )

            def bands_of_tail(tt):
                """Bands fully determined once ot[tt] exists."""
                lst = []
                if tt >= 1:
                    lst.append(4 * tt - 1)
                lst += [4 * tt, 4 * tt + 1, 4 * tt + 2]
                if tt == 31:
                    lst.append(127)
                return lst

            def norm_tail(nb_pv, ns, po):
                """normalize sub-tile ns of n-block nb_pv -> ot ring tile."""
                tt = 4 * nb_pv + ns
                rcp = sm_pool.tile([128, 1], f32, tag="rcp")
                nc.vector.reciprocal(rcp[:], po[:, 128:129])
                ot = sm_pool.tile([128, 128], bf16, tag="ot", name="ot_t")
                nc.vector.tensor_scalar(ot[:], po[:, 0:128], rcp[:], None, AOp.mult)
                ot_tiles[tt] = ot

            # ---- main loop ----
            # it(nb): PV(nb-1) + scores(nb) region-wise; bands of tiles
            # 4(nb-2)..4(nb-2)+3 interleaved between PV chunks; norms of
            # nb-1 at the end; q extraction for the next-next block last.
            for nb in range(NBLK):
                pos = None
                for b in (2 * nb + 2, 2 * nb + 3):
                    if b < NFB:
                        load_fw(b)
                if nb >= 1:
                    po0 = ps_o.tile([128, 2, 129], f32, tag="po")
                    po1 = ps_o.tile([128, 2, 129], f32, tag="po")
                    pos = (po0[:, 0, :], po0[:, 1, :], po1[:, 0, :], po1[:, 1, :])
                def pv_chunk(i):
                    for ns in range(4):
                        for mi in range(8 * i, 8 * i + 8):
                            nc.tensor.matmul(
                                pos[ns],
                                at[:, mi, 128 * ns : 128 * ns + 128],
                                kt1[:, mi, :],
                                start=(mi == 0),
                                stop=(mi == MI - 1),
                            )
                        if i == 3:
                            # normalize as soon as this sub-tile's m-sweep
                            # stops: frees the po slot well before the next
                            # iteration's PV needs it
                            norm_tail(nb - 1, ns, pos[ns])

                def band_chunk(i):
                    for m in bands_of_tail(4 * (nb - 2) + i):
                        emit_band(m)

                qt = q_tiles[nb]

                def spair(mp):
                    ps = ps_s.tile([128, 2, 512], f32, tag="ps")
                    for s in range(2):
                        mi = 2 * mp + s
                        nc.tensor.matmul(
                            ps[:, s, :],
                            k_bf[:, 128 * mi : 128 * mi + 128],
                            qt[:],
                            start=True,
                            stop=True,
                        )
                    nc.scalar.activation(
                        at[:, 2 * mp : 2 * mp + 2, :],
                        ps[:],
                        ActFn.Exp,
                        bias=0.0,
                        scale=EXP_SCALE,
                    )

                # software-pipelined: score pairs of region i straddle the PV
                # chunk of region i+1 so each exp gets PE-time as slack
                # before its ps_s slot is reused (ring is only 2 deep).
                if pos is not None:
                    pv_chunk(0)
                if nb >= 2:
                    band_chunk(0)
                for i in range(4):
                    spair(4 * i)
                    spair(4 * i + 1)
                    if i < 3 and pos is not None:
                        pv_chunk(i + 1)
                    spair(4 * i + 2)
                    spair(4 * i + 3)
                    if i < 3 and nb >= 2:
                        band_chunk(i + 1)
                for b in (2 * nb + 2, 2 * nb + 3):
                    if b < NFB:
                        q_extract(b)

            # ---- epilogue: PV(7) + bands of tiles 24..31 ----
            po0 = ps_o.tile([128, 2, 129], f32, tag="po")
            po1 = ps_o.tile([128, 2, 129], f32, tag="po")
            epos = (po0[:, 0, :], po0[:, 1, :], po1[:, 0, :], po1[:, 1, :])
            for i in range(4):
                bands = bands_of_tail(24 + i)
                for ns in range(4):
                    for mi in range(8 * i, 8 * i + 8):
                        nc.tensor.matmul(
                            epos[ns],
                            at[:, mi, 128 * ns : 128 * ns + 128],
                            kt1[:, mi, :],
                            start=(mi == 0),
                            stop=(mi == MI - 1),
                        )
                    if i == 3:
                        norm_tail(NBLK - 1, ns, epos[ns])
                    if bands:
                        emit_band(bands.pop(0))
                while bands:
                    emit_band(bands.pop(0))
            for ns in range(4):
                for m in bands_of_tail(28 + ns):
                    emit_band(m)

    nc.compile()
    return nc


_NC_CACHE = None


def _get_nc():
    global _NC_CACHE
    if _NC_CACHE is None:
        _NC_CACHE = build_kernel()
    return _NC_CACHE


def run(feat_wide: np.ndarray, feat_narrow: np.ndarray, trace: bool = False):
    """Run on 8 NeuronCores; returns (output [8,128,256,256], BassKernelResults)."""
    from concourse.bass_utils import run_bass_kernel_spmd
    import ml_dtypes

    B, C, H, W = feat_wide.shape
    assert (B, C, H, W) == (8, 128, 256, 256)

    bandmat = _build_band_matrices().astype(ml_dtypes.bfloat16)
    identity = np.eye(128, dtype=ml_dtypes.bfloat16)

    nc = _get_nc()
    in_maps = [
        {
            "feat_wide": np.ascontiguousarray(np.asarray(feat_wide[b], dtype=np.float32)),
            "feat_narrow": np.ascontiguousarray(np.asarray(feat_narrow[b], dtype=np.float32)),
            "bandmat": bandmat,
            "ident": identity,
        }
        for b in range(B)
    ]
    res = run_bass_kernel_spmd(nc, in_maps, core_ids=list(range(8)), trace=trace)
    out = np.stack([res.results[b]["out"] for b in range(B)], axis=0)
    return out, res


def kernel(feat_wide: np.ndarray, feat_narrow: np.ndarray) -> np.ndarray:
    out, _ = run(feat_wide, feat_narrow, trace=False)
    return out


# revision 3
# speedup vs baseline: 1.0206x; 1.0156x over previous
"""Trainium2 Bass kernel v4 for nn_AttentionFusion — fused stream, PE upsample.

Per batch element (B=8 -> one NeuronCore each):
    Q = ds(feat_wide), K = ds(feat_narrow)      # 2x2 center sums, [C, 4096]
    attn = softmax(Q^T K / (16 sqrt(C)))
    out = feat_wide + up4((attn @ K^T)^T)       # bilinear 4x upsample + residual

v4 layout: feat_wide streams through SBUF ONCE in 16-row blocks (80 MiB total
HBM traffic).  The whole upsample runs on the PE: for each normalized
attention sub-tile ot[tt] = [n=(2h,64w), c] the output rows are produced by
matmuls against 8 host-built [128, 512] matrices that fuse the 2-tap H-up
weights with the W-upsample matrix; bands of 2 output rows accumulate in PSUM
(cross-tile bands get one matmul from each neighbor tile), then a single DVE
add folds the streamed feat_wide rows in-place and the block is written out.

Engine mapping per n-block iteration (~22 us DMA slot):
  - PE: PV(nb-1) m-region-wise ahead of scores(nb) (single fp8 attn buffer,
    WAR-safe), band matmuls of tiles from nb-2 interleaved between regions
  - ScalarE: exp only (PSUM -> fp8 SBUF)
  - DVE: normalize (rcp + scale) + one [128,512] band add per 2 output rows
  - Pool: fw DMA waits + Q extraction (row-pair + col-pair sums)
  - Sync: fn chunk loads (prologue), fw block loads, out writes
"""

import math

import numpy as np


# ----------------------------------------------------------------------------
# numpy-side constants
# ----------------------------------------------------------------------------

def _build_upsample_matrix(n_in: int, n_out: int) -> np.ndarray:
    """U[h, H]: out[H] = sum_h U[h, H] * in[h] for torch-style bilinear,
    align_corners=False, antialias=False, scale n_out/n_in."""
    U = np.zeros((n_in, n_out), dtype=np.float64)
    scale = n_in / n_out
    for o in range(n_out):
        src = (o + 0.5) * scale - 0.5
        k0 = int(math.floor(src))
        frac = src - k0
        for k, wt in ((k0, 1.0 - frac), (k0 + 1, frac)):
            kc = min(max(k, 0), n_in - 1)
            U[kc, o] += wt
    return U


# Band weight matrices: band m covers output rows (2m, 2m+1).
#   m even (=2k): rows 4k+0,4k+1 = (0.375, 0.125) * y[k-1] + (0.625, 0.875) * y[k]
#   m odd (=2k+1): rows 4k+2,4k+3 = (0.875, 0.625) * y[k] + (0.125, 0.375) * y[k+1]
# y[k] lives in ot tile k//2 at h' = k%2.  Matrix M[(h',w), (r2,W)] =
# coef[h'][r2] * Uw[w, W] with Uw = 0.25 * U(64->256) (0.25 undoes the
# unscaled 2x2-sum downsample of K).
_BAND_COEFS = {
    "IN0": {0: (0.375, 0.125), 1: (0.625, 0.875)},  # m even, k odd: in-tile
    "IN1": {0: (0.875, 0.625), 1: (0.125, 0.375)},  # m odd, k even: in-tile
    "CEA": {1: (0.375, 0.125)},  # m even, k even: y[k-1] from tile tt-1
    "CEB": {0: (0.625, 0.875)},  # m even, k even: y[k] from tile tt
    "COA": {1: (0.875, 0.625)},  # m odd, k odd: y[k] from tile tt-1
    "COB": {0: (0.125, 0.375)},  # m odd, k odd: y[k+1] from tile tt
    "ET": {0: (1.0, 1.0)},       # band 0: rows 0,1 clamp to y[0]
    "EB": {1: (1.0, 1.0)},       # band 127: rows 254,255 clamp to y[63]
}
_BAND_NAMES = ("IN0", "IN1", "CEA", "CEB", "COA", "COB", "ET", "EB")


def _build_band_matrices() -> np.ndarray:
    """[8, 128, 512] f64: fused H-up x W-up band weight matrices."""
    Uw = _build_upsample_matrix(64, 256) * 0.25  # [64, 256]
    out = np.zeros((8, 128, 512), dtype=np.float64)
    for idx, name in enumerate(_BAND_NAMES):
        for hp, (c0, c1) in _BAND_COEFS[name].items():
            for r2, cf in ((0, c0), (1, c1)):
                out[idx, hp * 64 : hp * 64 + 64, r2 * 256 : r2 * 256 + 256] = (
                    cf * Uw
                )
    return out


# ----------------------------------------------------------------------------
# Bass kernel builder
# ----------------------------------------------------------------------------

def build_kernel():
    import concourse.bacc as bacc
    import concourse.bass as bass
    import concourse.mybir as mybir
    from concourse import tile

    f32 = mybir.dt.float32
    bf16 = mybir.dt.bfloat16
    f8 = mybir.dt.float8e4
    AOp = mybir.AluOpType
    ActFn = mybir.ActivationFunctionType

    C = 128          # channels = partitions
    HW = 256         # full resolution
    hw = 64          # downsampled resolution
    N = hw * hw      # 4096 attention positions
    MI = 32          # m tiles of 128
    NBLK = 8         # n blocks of 512
    NFB = 16         # fw row blocks of 16 rows
    EXP_SCALE = 1.0 / (16.0 * math.sqrt(C))
    MIDX = {n: i for i, n in enumerate(_BAND_NAMES)}

    nc = bacc.Bacc("TRN2", target_bir_lowering=False, debug=False)

    fw = nc.dram_tensor("feat_wide", [C, HW, HW], f32, kind="ExternalInput")
    fn = nc.dram_tensor("feat_narrow", [C, HW, HW], f32, kind="ExternalInput")
    bm = nc.dram_tensor("bandmat", [8, 128, 512], bf16, kind="ExternalInput")
    ident = nc.dram_tensor("ident", [128, 128], bf16, kind="ExternalInput")
    out = nc.dram_tensor("out", [C, HW, HW], f32, kind="ExternalOutput")

    with tile.TileContext(nc) as tc:
        with (
            tc.tile_pool(name="const", bufs=1) as const_pool,
            tc.tile_pool(name="big", bufs=1) as big_pool,
            tc.tile_pool(name="fwp", bufs=8) as fw_pool,
            tc.tile_pool(name="kch", bufs=2) as kch_pool,
            tc.tile_pool(name="qb", bufs=2) as qb_pool,
            tc.tile_pool(name="rs", bufs=2) as rs_pool,
            tc.tile_pool(name="sm", bufs=6) as sm_pool,
            tc.tile_pool(name="ps_s", bufs=2, space=bass.MemorySpace.PSUM) as ps_s,
            tc.tile_pool(name="ps_o", bufs=2, space=bass.MemorySpace.PSUM) as ps_o,
            tc.tile_pool(name="ps_b", bufs=2, space=bass.MemorySpace.PSUM) as ps_b,
        ):
            # ---- constants ----
            bm_t = const_pool.tile([128, 8, 512], bf16)
            for j in range(8):
                nc.sync.dma_start(bm_t[:, j, :], bm.ap()[j, :, :])
            id_t = const_pool.tile([128, 128], bf16)
            nc.sync.dma_start(id_t[:], ident[:, :])

            # ---- persistent buffers ----
            k_bf = big_pool.tile([C, N], bf16)
            kt1 = big_pool.tile([128, MI, 129], f8)   # K^T tiles + ones col
            nc.vector.memset(kt1[:], 1.0)
            at = big_pool.tile([128, MI, 512], f8)    # attn^T, single buffer

            fn3 = fn.ap().rearrange("c (i r) w -> c i (r w)", r=4)

            fw_tiles = {}
            q_tiles = {}
            ot_tiles = {}

            def load_fw(b):
                """Stream in fw rows 16b..16b+16 (sync queue dispatch)."""
                t = fw_pool.tile([C, 16, HW], f32, tag="fw")
                nc.sync.dma_start(t[:], fw.ap()[:, 16 * b : 16 * b + 16, :])
                fw_tiles[b] = t

            def q_extract(b):
                """Q columns of fw block b: rows 4i+1,4i+2, cols 4j+1,4j+2.
                Entirely on Pool so the DMA wait never blocks DVE."""
                t = fw_tiles[b]
                nbq, half = divmod(b, 2)
                if half == 0:
                    q_tiles[nbq] = qb_pool.tile([C, 512], bf16, tag="q", name="qt")
                qt = q_tiles[nbq]
                fw4 = t[:].rearrange("c (j r) w -> c j r w", r=4)
                rsum = rs_pool.tile([C, 4, 256], bf16, tag="qrs")
                nc.gpsimd.tensor_tensor(
                    rsum[:], fw4[:, :, 1, :], fw4[:, :, 2, :], AOp.add
                )
                rs4 = rsum[:].rearrange("c j (k f) -> c j k f", f=4)
                qv = qt[:, half * 256 : half * 256 + 256].rearrange(
                    "c (j k) -> c j k", k=hw
                )
                nc.gpsimd.tensor_tensor(qv, rs4[:, :, :, 1], rs4[:, :, :, 2], AOp.add)

            def load_k_chunk(cc):
                """fn rows 4i+1,4i+2 for i=4cc..4cc+4 -> k_bf cols + kt1 tiles."""
                ch = kch_pool.tile([C, 4, 512], f32, tag="kch")
                nc.sync.dma_start(ch[:], fn3[:, 4 * cc : 4 * cc + 4, 256:768])
                ch2 = ch[:].rearrange("c i (r w) -> c i r w", r=2)
                rsum = rs_pool.tile([C, 4, 256], bf16, tag="krs")
                nc.vector.tensor_tensor(rsum[:], ch2[:, :, 0, :], ch2[:, :, 1, :], AOp.add)
                rs4 = rsum[:].rearrange("c j (k f) -> c j k f", f=4)
                kv = k_bf[:, 256 * cc : 256 * cc + 256].rearrange(
                    "c (j k) -> c j k", k=hw
                )
                nc.vector.tensor_tensor(kv, rs4[:, :, :, 1], rs4[:, :, :, 2], AOp.add)
                for mi in (2 * cc, 2 * cc + 1):
                    pt = ps_b.tile([128, 512], f32, tag="bd")
                    ptb = pt[:, 0:64].bitcast(bf16)  # [128, 128] bf16 view
                    nc.tensor.transpose(
                        ptb, k_bf[:, 128 * mi : 128 * mi + 128], id_t[:]
                    )
                    nc.vector.tensor_copy(kt1[:, mi, 0:128], ptb)

            # ---- prologue: fw first (q gates the first scores), then K ----
            load_fw(0)
            load_fw(1)
            for cc in range(16):
                load_k_chunk(cc)
            q_extract(0)
            q_extract(1)

            def band_mm_list(m):
                """(ot tile, matrix name) pairs accumulating band m."""
                if m == 0:
                    return ((0, "ET"),)
                if m == 127:
                    return ((31, "EB"),)
                if m % 2 == 0:
                    k = m // 2
                    if k % 2 == 1:
                        return (((k - 1) // 2, "IN0"),)
                    return ((k // 2 - 1, "CEA"), (k // 2, "CEB"))
                k = (m - 1) // 2
                if k % 2 == 0:
                    return ((k // 2, "IN1"),)
                return (((k - 1) // 2, "COA"), ((k + 1) // 2, "COB"))

            def emit_band_mms(m, bd):
                mms = band_mm_list(m)
                for j, (tt, name) in enumerate(mms):
                    nc.tensor.matmul(
                        bd,
                        ot_tiles[tt][:],
                        bm_t[:, MIDX[name], :],
                        start=(j == 0),
                        stop=(j == len(mms) - 1),
                    )

            def emit_band(m):
                """Band m = output rows (2m, 2m+1): PE matmul(s) into PSUM,
                DVE add of streamed fw rows in place, write block when done."""
                bd = ps_b.tile([128, 512], f32, tag="bd")
                emit_band_mms(m, bd[:])
                b = m // 8
                lr = 2 * (m % 8)
                fwb = fw_tiles[b]
                nc.vector.tensor_tensor(
                    fwb[:, lr : lr + 2, :], bd[:], fwb[:, lr : lr + 2, :], AOp.add
                )
                if m % 8 == 7:
                    nc.sync.dma_start(
                        out.ap()[:, 16 * b : 16 * b + 16, :], fw_tiles.pop(b)[:]
                    )

            def bands_of_tail(tt):
                """Bands fully determined once ot[tt] exists."""
                lst = []
                if tt >= 1:
                    lst.append(4 * tt - 1)
                lst += [4 * tt, 4 * tt + 1, 4 * tt + 2]
                if tt == 31:
                    lst.append(127)
                return lst

            def norm_tail(nb_pv, ns, po):
                """normalize sub-tile ns of n-block nb_pv -> ot ring tile."""
                tt = 4 * nb_pv + ns
                rcp = sm_pool.tile([128, 1], f32, tag="rcp")
                nc.vector.reciprocal(rcp[:], po[:, 128:129])
                ot = sm_pool.tile([128, 128], bf16, tag="ot", name="ot_t")
                nc.vector.tensor_scalar(ot[:], po[:, 0:128], rcp[:], None, AOp.mult)
                ot_tiles[tt] = ot

            # ---- main loop ----
            # it(nb): PV(nb-1) + scores(nb) region-wise; bands of tiles
            # 4(nb-2)..4(nb-2)+3 interleaved between PV chunks; norms of
            # nb-1 at the end; q extraction for the next-next block last.
            for nb in range(NBLK):
                pos = None
                for b in (2 * nb + 2, 2 * nb + 3):
                    if b < NFB:
                        load_fw(b)
                if nb >= 1:
                    po0 = ps_o.tile([128, 2, 129], f32, tag="po")
                    po1 = ps_o.tile([128, 2, 129], f32, tag="po")
                    pos = (po0[:, 0, :], po0[:, 1, :], po1[:, 0, :], po1[:, 1, :])
                def pv_part(mi0, mi1):
                    for ns in range(4):
                        for mi in range(mi0, mi1):
                            nc.tensor.matmul(
                                pos[ns],
                                at[:, mi, 128 * ns : 128 * ns + 128],
                                kt1[:, mi, :],
                                start=(mi == 0),
                                stop=(mi == MI - 1),
                            )

                def band_chunk(i):
                    for m in bands_of_tail(4 * (nb - 2) + i):
                        emit_band(m)

                qt = q_tiles[nb]

                def spair(mp):
                    ps = ps_s.tile([128, 2, 512], f32, tag="ps")
                    for s in range(2):
                        mi = 2 * mp + s
                        nc.tensor.matmul(
                            ps[:, s, :],
                            k_bf[:, 128 * mi : 128 * mi + 128],
                            qt[:],
                            start=True,
                            stop=True,
                        )
                    nc.scalar.activation(
                        at[:, 2 * mp : 2 * mp + 2, :],
                        ps[:],
                        ActFn.Exp,
                        bias=0.0,
                        scale=EXP_SCALE,
                    )

                # fine-grained weave: per m-pair, its PV matmuls (for the
                # m-tiles the following scores will overwrite), then its
                # scores + exp, then one band of the nb-2 output stream.
                # Exps (the ScalarE pacer) start ~1 us into the iteration
                # and stream continuously; each band's PE<->DVE round trip
                # gets a whole m-pair slot of slack before its 2-deep PSUM
                # ring slot is reused.
                bandq = []
                if nb >= 2:
                    for i in range(4):
                        bandq += list(bands_of_tail(4 * (nb - 2) + i))
                for mp in range(16):
                    if pos is not None:
                        pv_part(2 * mp, 2 * mp + 2)
                        if mp == 15:
                            for ns in range(4):
                                norm_tail(nb - 1, ns, pos[ns])
                    spair(mp)
                    if bandq:
                        emit_band(bandq.pop(0))
                while bandq:
                    emit_band(bandq.pop(0))
                for b in (2 * nb + 2, 2 * nb + 3):
                    if b < NFB:
                        q_extract(b)

            # ---- epilogue: PV(7) + bands of tiles 24..31 ----
            po0 = ps_o.tile([128, 2, 129], f32, tag="po")
            po1 = ps_o.tile([128, 2, 129], f32, tag="po")
            epos = (po0[:, 0, :], po0[:, 1, :], po1[:, 0, :], po1[:, 1, :])
            for i in range(4):
                bands = bands_of_tail(24 + i)
                for ns in range(4):
                    for mi in range(8 * i, 8 * i + 8):
                        nc.tensor.matmul(
                            epos[ns],
                            at[:, mi, 128 * ns : 128 * ns + 128],
                            kt1[:, mi, :],
                            start=(mi == 0),
                            stop=(mi == MI - 1),
                        )
                    if i == 3:
                        norm_tail(NBLK - 1, ns, epos[ns])
                    if bands:
                        emit_band(bands.pop(0))
                while bands:
                    emit_band(bands.pop(0))
            # tail bands 111..127: 111 completes block 13; pair the rest into
            # 2-band ps_s tiles (scores are done, those banks are free) so
            # one DVE add covers 4 output rows — halves PE<->DVE round trips
            emit_band(111)
            for m0 in range(112, 128, 2):
                bd2 = ps_s.tile([128, 2, 512], f32, tag="ps")
                emit_band_mms(m0, bd2[:, 0, :])
                emit_band_mms(m0 + 1, bd2[:, 1, :])
                b = m0 // 8
                lr = 2 * (m0 % 8)
                fwb = fw_tiles[b]
                nc.vector.tensor_tensor(
                    fwb[:, lr : lr + 4, :], bd2[:], fwb[:, lr : lr + 4, :], AOp.add
                )
                if (m0 + 1) % 8 == 7:
                    nc.sync.dma_start(
                        out.ap()[:, 16 * b : 16 * b + 16, :], fw_tiles.pop(b)[:]
                    )

    nc.compile()
    return nc


_NC_CACHE = None


def _get_nc():
    global _NC_CACHE
    if _NC_CACHE is None:
        _NC_CACHE = build_kernel()
    return _NC_CACHE


def run(feat_wide: np.ndarray, feat_narrow: np.ndarray, trace: bool = False):
    """Run on 8 NeuronCores; returns (output [8,128,256,256], BassKernelResults)."""
    from concourse.bass_utils import run_bass_kernel_spmd
    import ml_dtypes

    B, C, H, W = feat_wide.shape
    assert (B, C, H, W) == (8, 128, 256, 256)

    bandmat = _build_band_matrices().astype(ml_dtypes.bfloat16)
    identity = np.eye(128, dtype=ml_dtypes.bfloat16)

    nc = _get_nc()
    in_maps = [
        {
            "feat_wide": np.ascontiguousarray(np.asarray(feat_wide[b], dtype=np.float32)),
            "feat_narrow": np.ascontiguousarray(np.asarray(feat_narrow[b], dtype=np.float32)),
            "bandmat": bandmat,
            "ident": identity,
        }
        for b in range(B)
    ]
    res = run_bass_kernel_spmd(nc, in_maps, core_ids=list(range(8)), trace=trace)
    out = np.stack([res.results[b]["out"] for b in range(B)], axis=0)
    return out, res


def kernel(feat_wide: np.ndarray, feat_narrow: np.ndarray) -> np.ndarray:
    out, _ = run(feat_wide, feat_narrow, trace=False)
    return out
